# revision 5
# baseline (speedup 1.0000x reference)
"""Trainium2 Bass kernel for nn_ExpFilter: y = Linear(x); v = EMA_t(y).

Strategy (per core, batch-sharded 8 ways -> B_local=4):
  rows = T*B_local = 16384 flattened (t-major, b-minor), 128 blocks of 128
  rows (= 32 timesteps x 4 batch).  The EMA along t is folded into PE
  matmuls: with p = 0.1, p^k decays below f16 resolution in ~7 steps, so
  the scan is exactly (to fp32 eps) a banded lower-triangular matmul A
  touching only the current block and the tail of the previous block.

  Per block:
    U^T[i, t] = sum_j X[j, i] * A0[j, t]  (+ X_prev[j, i] * A1[j, t])
      -> 8 f16 matmuls, N=128/32, lhsT = X block chunks (natural layout!
         this fuses the X transpose into the filter matmul for free)
    V[t, o]  = sum_i U^T[i, t] * W^T[i, o]   -> 4 f16 matmuls, N=512
    V += c[t] * bias   (DVE tensor_add with a precomputed bias tile)

  HBM traffic/core = 32 MiB in + 32 MiB out @ ~358 GB/s  => ~187 us
  PE/core ~ 12 MM/block * 128 blocks                     => ~173 us
"""
import sys

sys.path.insert(0, "/opt/trn_rl_repo")

import numpy as np

import concourse.bass as bass
import concourse.tile as tile
from concourse import bacc, mybir
from concourse.bass_utils import run_bass_kernel_spmd

P = 0.1
T, B, I, O = 4096, 32, 512, 512
NCORES = 8
BL = B // NCORES          # 4 local batches
BS = BL                   # rows per timestep in flattened layout
ROWS = T * BL             # 16384
BLK = 128                 # rows per block
NBLK = ROWS // BLK        # 128
SLAB = 8                  # blocks per slab (DMA granularity: 2 MiB)
NSLAB = NBLK // SLAB      # 16

TRACE = False             # set by test harness for profiling runs
OUT_F16 = False           # write the output as f16 (halves store traffic)

_cache = {}


def _build_nc():
    f32 = mybir.dt.float32
    f16 = mybir.dt.float16
    nc = bacc.Bacc("TRN2", target_bir_lowering=False, debug=False,
                   num_devices=NCORES)
    x = nc.dram_tensor("x", [ROWS, I], f32, kind="ExternalInput")
    wt = nc.dram_tensor("wt", [128, 4, O], f16, kind="ExternalInput")
    a0 = nc.dram_tensor("a0", [BLK, BLK], f16, kind="ExternalInput")
    a1 = nc.dram_tensor("a1", [BLK, 32], f16, kind="ExternalInput")
    b0 = nc.dram_tensor("b0", [BLK, O], f32, kind="ExternalInput")
    bs = nc.dram_tensor("bs", [BLK, O], f32, kind="ExternalInput")
    out_dt = f16 if OUT_F16 else f32
    y = nc.dram_tensor("y", [ROWS, O], out_dt, kind="ExternalOutput")

    ts = bass.ts
    with tile.TileContext(nc) as tc:
        with (
            tc.tile_pool(name="const", bufs=1) as cpool,
            tc.tile_pool(name="xin", bufs=2) as px,
            tc.tile_pool(name="x16", bufs=3) as px16,
            tc.tile_pool(name="u16", bufs=3) as pu16,
            tc.tile_pool(name="out", bufs=2) as pout,
            tc.tile_pool(name="psu", bufs=2, space=bass.MemorySpace.PSUM) as ppsu,
            tc.tile_pool(name="psv", bufs=2, space=bass.MemorySpace.PSUM) as ppsv,
        ):
            wt_sb = cpool.tile([128, 4, O], f16)
            nc.sync.dma_start(wt_sb[:], wt.ap())
            a0_sb = cpool.tile([BLK, BLK], f16)
            nc.sync.dma_start(a0_sb[:], a0.ap())
            a1_sb = cpool.tile([BLK, 32], f16)
            nc.sync.dma_start(a1_sb[:], a1.ap())
            b0_sb = cpool.tile([BLK, O], f32)
            nc.sync.dma_start(b0_sb[:], b0.ap())
            bs_sb = cpool.tile([BLK, O], f32)
            nc.sync.dma_start(bs_sb[:], bs.ap())

            xr = x.ap().rearrange("(S s p) i -> p S s i", p=128, s=SLAB)
            yr = y.ap().rearrange("(S s p) o -> p S s o", p=128, s=SLAB)

            x16_prev = None
            for sl in range(NSLAB):
                xf = px.tile([128, SLAB, I], f32)
                nc.sync.dma_start(xf[:], xr[:, sl, :, :])
                x16 = px16.tile([128, SLAB, I], f16)
                nc.scalar.copy(x16[:], xf[:])
                of = pout.tile([128, SLAB, O], out_dt)
                for b in range(SLAB):
                    blk = sl * SLAB + b
                    pu = ppsu.tile([128, 4, BLK], f32)
                    for c in range(4):
                        lhs0 = x16[:, b, ts(c, 128)]
                        if blk == 0:
                            nc.tensor.matmul(pu[:, c, :], lhs0, a0_sb[:],
                                             start=True, stop=True)
                        else:
                            prev = (x16[:, b - 1, ts(c, 128)] if b > 0
                                    else x16_prev[:, SLAB - 1, ts(c, 128)])
                            nc.tensor.matmul(pu[:, c, :], lhs0, a0_sb[:],
                                             start=True, stop=False)
                            nc.tensor.matmul(pu[:, c, 0:32], prev, a1_sb[:],
                                             start=False, stop=True)
                    u16 = pu16.tile([128, 4, BLK], f16)
                    nc.scalar.copy(u16[:], pu[:])
                    pv = ppsv.tile([128, O], f32)
                    for c in range(4):
                        nc.tensor.matmul(pv[:], u16[:, c, :], wt_sb[:, c, :],
                                         start=(c == 0), stop=(c == 3))
                    bias_sb = b0_sb if blk == 0 else bs_sb
                    nc.vector.tensor_add(of[:, b, :], pv[:], bias_sb[:])
                nc.sync.dma_start(yr[:, sl, :, :], of[:])
                x16_prev = x16
    nc.compile()
    return nc


def _constants(weight, bias):
    # W^T chunks: wt[k, c, o] = W[o, 128c + k]
    wt16 = np.ascontiguousarray(
        weight.T.reshape(4, 128, O).transpose(1, 0, 2)).astype(np.float16)
    a0 = np.zeros((BLK, BLK), np.float64)
    for j in range(BLK):
        for t in range(j, BLK, BS):
            a0[j, t] = P ** ((t - j) // BS)
    a1 = np.zeros((BLK, 32), np.float64)
    for j in range(BLK):
        for t in range(32):
            lag = t + BLK - j
            if lag % BS == 0:
                a1[j, t] = P ** (lag // BS)
    a0 = a0.astype(np.float16)
    a1 = a1.astype(np.float16)
    b0 = np.empty((BLK, O), np.float32)
    for r in range(BLK):
        c = (1.0 - P ** (r // BS + 1)) / (1.0 - P)
        b0[r] = bias * c
    bsS = np.tile((bias / (1.0 - P)).astype(np.float32), (BLK, 1))
    return wt16, a0, a1, b0, bsS


def kernel(input_tensor, weight, bias):
    input_tensor = np.asarray(input_tensor, dtype=np.float32)
    weight = np.asarray(weight, dtype=np.float32)
    bias = np.asarray(bias, dtype=np.float32)

    if "nc" not in _cache:
        _cache["nc"] = _build_nc()
    nc = _cache["nc"]

    wt16, a0, a1, b0, bsS = _constants(weight, bias)
    in_maps = []
    for c in range(NCORES):
        xc = np.ascontiguousarray(
            input_tensor[:, c * BL:(c + 1) * BL, :]).reshape(ROWS, I)
        in_maps.append({"x": xc, "wt": wt16, "a0": a0, "a1": a1,
                        "b0": b0, "bs": bsS})

    res = run_bass_kernel_spmd(nc, in_maps, core_ids=list(range(NCORES)),
                               trace=TRACE)
    _cache["last_result"] = res

    out = np.empty((T, B, O), np.float32)
    for c in range(NCORES):
        yc = res.results[c]["y"].astype(np.float32, copy=False)
        out[:, c * BL:(c + 1) * BL, :] = yc.reshape(T, BL, O)
    return out


# revision 6
# speedup vs baseline: 1.2006x; 1.2006x over previous
"""Trainium2 Bass kernel for nn_ExpFilter: y = Linear(x); v = EMA_t(y).

Strategy (per core, batch-sharded 8 ways -> B_local=4):
  rows = T*B_local = 16384 flattened (t-major, b-minor), 128 blocks of 128
  rows (= 32 timesteps x 4 batch).  The EMA along t is folded into PE
  matmuls: with p = 0.1, p^k decays below f16 resolution in ~7 steps, so
  the scan is exactly (to fp32 eps) a banded lower-triangular matmul A
  touching only the current block and the tail of the previous block.

  Per block:
    U^T[i, t] = sum_j X[j, i] * A0[j, t]  (+ X_prev[j, i] * A1[j, t])
      -> 8 f16 matmuls, N=128/32, lhsT = X block chunks (natural layout!
         this fuses the X transpose into the filter matmul for free)
    V[t, o]  = sum_i U^T[i, t] * W^T[i, o]   -> 4 f16 matmuls, N=512
    V += c[t] * bias   (DVE tensor_add with a precomputed bias tile)

  HBM traffic/core = 32 MiB in + 32 MiB out @ ~358 GB/s  => ~187 us
  PE/core ~ 12 MM/block * 128 blocks                     => ~173 us
"""
import sys

sys.path.insert(0, "/opt/trn_rl_repo")

import numpy as np

import concourse.bass as bass
import concourse.tile as tile
from concourse import bacc, mybir
from concourse.bass_utils import run_bass_kernel_spmd

P = 0.1
T, B, I, O = 4096, 32, 512, 512
NCORES = 8
BL = B // NCORES          # 4 local batches
BS = BL                   # rows per timestep in flattened layout
ROWS = T * BL             # 16384
BLK = 128                 # rows per block
NBLK = ROWS // BLK        # 128
SLAB = 4                  # blocks per slab (DMA granularity: 1 MiB)
NSLAB = NBLK // SLAB      # 32

TRACE = False             # set by test harness for profiling runs
OUT_F16 = True           # write the output as f16 (halves store traffic)

_cache = {}


def _build_nc():
    f32 = mybir.dt.float32
    f16 = mybir.dt.float16
    nc = bacc.Bacc("TRN2", target_bir_lowering=False, debug=False,
                   num_devices=NCORES)
    x = nc.dram_tensor("x", [ROWS, I], f32, kind="ExternalInput")
    wt = nc.dram_tensor("wt", [128, 4, O], f16, kind="ExternalInput")
    a0 = nc.dram_tensor("a0", [BLK, BLK], f16, kind="ExternalInput")
    a1 = nc.dram_tensor("a1", [BLK, 32], f16, kind="ExternalInput")
    b0 = nc.dram_tensor("b0", [BLK, O], f32, kind="ExternalInput")
    bs = nc.dram_tensor("bs", [BLK, O], f32, kind="ExternalInput")
    out_dt = f16 if OUT_F16 else f32
    y = nc.dram_tensor("y", [ROWS, O], out_dt, kind="ExternalOutput")

    ts = bass.ts
    with tile.TileContext(nc) as tc:
        with (
            tc.tile_pool(name="const", bufs=1) as cpool,
            tc.tile_pool(name="xin", bufs=2) as px,
            tc.tile_pool(name="x16", bufs=3) as px16,
            tc.tile_pool(name="u16", bufs=3) as pu16,
            tc.tile_pool(name="out", bufs=2) as pout,
            tc.tile_pool(name="psu", bufs=2, space=bass.MemorySpace.PSUM) as ppsu,
            tc.tile_pool(name="psv", bufs=2, space=bass.MemorySpace.PSUM) as ppsv,
        ):
            wt_sb = cpool.tile([128, 4, O], f16)
            nc.sync.dma_start(wt_sb[:], wt.ap())
            a0_sb = cpool.tile([BLK, BLK], f16)
            nc.sync.dma_start(a0_sb[:], a0.ap())
            a1_sb = cpool.tile([BLK, 32], f16)
            nc.sync.dma_start(a1_sb[:], a1.ap())
            b0_sb = cpool.tile([BLK, O], f32)
            nc.sync.dma_start(b0_sb[:], b0.ap())
            bs_sb = cpool.tile([BLK, O], f32)
            nc.sync.dma_start(bs_sb[:], bs.ap())

            xr = x.ap().rearrange("(S s p) i -> p S s i", p=128, s=SLAB)
            yr = y.ap().rearrange("(S s p) o -> p S s o", p=128, s=SLAB)

            x16_prev = None
            for sl in range(NSLAB):
                xf = px.tile([128, SLAB, I], f32)
                nc.sync.dma_start(xf[:], xr[:, sl, :, :])
                x16 = px16.tile([128, SLAB, I], f16)
                nc.scalar.copy(x16[:], xf[:])
                of = pout.tile([128, SLAB, O], out_dt)
                for b in range(SLAB):
                    blk = sl * SLAB + b
                    pu = ppsu.tile([128, 4, BLK], f32)
                    for c in range(4):
                        lhs0 = x16[:, b, ts(c, 128)]
                        if blk == 0:
                            nc.tensor.matmul(pu[:, c, :], lhs0, a0_sb[:],
                                             start=True, stop=True)
                        else:
                            prev = (x16[:, b - 1, ts(c, 128)] if b > 0
                                    else x16_prev[:, SLAB - 1, ts(c, 128)])
                            nc.tensor.matmul(pu[:, c, :], lhs0, a0_sb[:],
                                             start=True, stop=False)
                            nc.tensor.matmul(pu[:, c, 0:32], prev, a1_sb[:],
                                             start=False, stop=True)
                    u16 = pu16.tile([128, 4, BLK], f16)
                    nc.scalar.copy(u16[:], pu[:])
                    pv = ppsv.tile([128, O], f32)
                    for c in range(4):
                        nc.tensor.matmul(pv[:], u16[:, c, :], wt_sb[:, c, :],
                                         start=(c == 0), stop=(c == 3))
                    bias_sb = b0_sb if blk == 0 else bs_sb
                    nc.vector.tensor_add(of[:, b, :], pv[:], bias_sb[:])
                nc.sync.dma_start(yr[:, sl, :, :], of[:])
                x16_prev = x16
    nc.compile()
    return nc


def _constants(weight, bias):
    # W^T chunks: wt[k, c, o] = W[o, 128c + k]
    wt16 = np.ascontiguousarray(
        weight.T.reshape(4, 128, O).transpose(1, 0, 2)).astype(np.float16)
    a0 = np.zeros((BLK, BLK), np.float64)
    for j in range(BLK):
        for t in range(j, BLK, BS):
            a0[j, t] = P ** ((t - j) // BS)
    a1 = np.zeros((BLK, 32), np.float64)
    for j in range(BLK):
        for t in range(32):
            lag = t + BLK - j
            if lag % BS == 0:
                a1[j, t] = P ** (lag // BS)
    a0 = a0.astype(np.float16)
    a1 = a1.astype(np.float16)
    b0 = np.empty((BLK, O), np.float32)
    for r in range(BLK):
        c = (1.0 - P ** (r // BS + 1)) / (1.0 - P)
        b0[r] = bias * c
    bsS = np.tile((bias / (1.0 - P)).astype(np.float32), (BLK, 1))
    return wt16, a0, a1, b0, bsS


def kernel(input_tensor, weight, bias):
    input_tensor = np.asarray(input_tensor, dtype=np.float32)
    weight = np.asarray(weight, dtype=np.float32)
    bias = np.asarray(bias, dtype=np.float32)

    if "nc" not in _cache:
        _cache["nc"] = _build_nc()
    nc = _cache["nc"]

    wt16, a0, a1, b0, bsS = _constants(weight, bias)
    in_maps = []
    for c in range(NCORES):
        xc = np.ascontiguousarray(
            input_tensor[:, c * BL:(c + 1) * BL, :]).reshape(ROWS, I)
        in_maps.append({"x": xc, "wt": wt16, "a0": a0, "a1": a1,
                        "b0": b0, "bs": bsS})

    res = run_bass_kernel_spmd(nc, in_maps, core_ids=list(range(NCORES)),
                               trace=TRACE)
    _cache["last_result"] = res

    out = np.empty((T, B, O), np.float32)
    for c in range(NCORES):
        yc = res.results[c]["y"].astype(np.float32, copy=False)
        out[:, c * BL:(c + 1) * BL, :] = yc.reshape(T, BL, O)
    return out
